# revision 21
# baseline (speedup 1.0000x reference)
"""Trainium2 Bass kernel for nn_CRANModel (CRAN-style memory recurrence).

Strategy
--------
The cache *keys* depend only on the token embeddings (new_key = mean_b(x) @ Wk),
so every step's attention scores, top-8 selection and softmax weights are
precomputable in one batched phase.  Only the *value* path is serial, and it
reduces algebraically to

    h_t = tanh(U_t + A_t @ Gd),     Gd_j = (1^T h_j) @ C' - D0_j

with   U      = [X | R_full] @ Wh + bh          (batched)
       R_full = Wfull @ values0                 (batched, Wfull = scattered
                                                 top-8 softmax weights)
       A      = Wfull[:, :64] * (slot < t)      (batched)
       C'     = Wv @ Wh_r / B,  D0 = values0[:64] @ Wh_r.

Phases 0 (precompute) and 1 (64-step scan) are replicated on all 8 cores
(no collectives); phase 2 (the 262 MB logits = hidden @ Wout projection,
the memory roofline) is sharded over the vocab dimension.

Matmul dtypes: the score path runs in true fp32 (top-8 selection is
sensitive to score perturbations); everything else runs in float32r
(fp32 with 11-bit mantissa, full-rate on the PE).  U is staged through
DRAM and streamed back during the scan, which also puts every scan rhs
at partition offset 0.
"""

import sys
import numpy as np

for p in ("/opt/trn_rl_repo", "/root/.axon_site/_ro/trn_rl_repo"):
    if p not in sys.path:
        sys.path.append(p)

# problem dims (hardcoded per contract)
T, B, V, E, H, N, DK, DV = 64, 32, 32000, 512, 512, 512, 256, 512
K = 8
NCORES = 8
VSH = V // NCORES            # 4000 vocab columns per core
TB = T * B                   # 2048 rows
RG = TB // 128               # 16 row groups of 128
VCH = (VSH + 127) // 128     # 32 v-chunks per core (last is ragged: 32 rows)
VLAST = VSH - (VCH - 1) * 128
_DEBUG = False               # add intermediate-tensor outputs for bisection


def _round_f32r(a):
    """Round-to-nearest-even to 11 explicit mantissa bits (fp32r)."""
    u = np.ascontiguousarray(a, np.float32).view(np.uint32)
    u = (u + 0x7FF + ((u >> 12) & 1)) & np.uint32(0xFFFFF000)
    return u.view(np.float32)


def _build_program():
    import contextlib
    import concourse.bass as bass
    import concourse.mybir as mybir
    import concourse.tile as tile
    from concourse import bacc
    from concourse.masks import make_identity

    f32 = mybir.dt.float32
    f32r = mybir.dt.float32r
    ACT = mybir.ActivationFunctionType

    nc = bacc.Bacc("TRN2", debug=False, target_bir_lowering=False)

    # ---------------- DRAM I/O ----------------
    d_tok = nc.dram_tensor("tok", [128, RG], mybir.dt.int32, kind="ExternalInput").ap()
    d_emb = nc.dram_tensor("emb", [V, E], f32, kind="ExternalInput").ap()
    d_wq = nc.dram_tensor("wq", [E, DK], f32, kind="ExternalInput").ap()
    d_wk = nc.dram_tensor("wk", [E, DK], f32, kind="ExternalInput").ap()
    d_k0T = nc.dram_tensor("k0T", [DK, N], f32, kind="ExternalInput").ap()
    d_wh = nc.dram_tensor("wh", [E + DV, H], f32r, kind="ExternalInput").ap()
    d_wvT = nc.dram_tensor("wvT", [DV, H], f32r, kind="ExternalInput").ap()
    d_v0 = nc.dram_tensor("v0", [N, DV], f32r, kind="ExternalInput").ap()
    d_v0hT = nc.dram_tensor("v0hT", [DV, T], f32r, kind="ExternalInput").ap()
    d_bhb = nc.dram_tensor("bhb", [128, H], f32, kind="ExternalInput").ap()
    d_maskRM = nc.dram_tensor("maskRM", [128, RG, T], f32, kind="ExternalInput").ap()
    d_negI = nc.dram_tensor("negI", [T, T], f32r, kind="ExternalInput").ap()
    d_eyer = nc.dram_tensor("eyer", [1, T * T], f32r, kind="ExternalInput").ap()
    d_wout = nc.dram_tensor("woutc", [VCH, 4, 128, 128], f32r,
                            kind="ExternalInput").ap()
    d_boutT = nc.dram_tensor("boutc", [128, VCH], f32, kind="ExternalInput").ap()
    d_ub = nc.dram_tensor("ubase", [T, B, H], f32r,
                          kind="ExternalOutput" if _DEBUG else "Internal").ap()
    d_out = nc.dram_tensor("out", [VSH, TB], f32, kind="ExternalOutput").ap()
    if _DEBUG:
        d_dbg_hT = nc.dram_tensor("dbg_hT", [128, 4, TB], f32,
                                  kind="ExternalOutput").ap()
        d_dbg_at = nc.dram_tensor("dbg_at", [T, TB], f32,
                                  kind="ExternalOutput").ap()
        d_dbg_gd = nc.dram_tensor("dbg_gd", [T, H], f32,
                                  kind="ExternalOutput").ap()
        d_dbg_s = nc.dram_tensor("dbg_s", [128, RG, N], f32,
                                 kind="ExternalOutput").ap()
        d_dbg_w = nc.dram_tensor("dbg_w", [128, RG, N], f32,
                                 kind="ExternalOutput").ap()

    with tile.TileContext(nc) as tc:
        with contextlib.ExitStack() as stack:
            cst = stack.enter_context(tc.tile_pool(name="cst", bufs=1))

            ident = cst.tile([128, 128], f32)
            make_identity(nc, ident)
            identr = cst.tile([B, B], f32r)
            nc.scalar.copy(out=identr[:], in_=ident[0:B, 0:B])

            tok_sb = cst.tile([128, RG], mybir.dt.int32)
            nc.sync.dma_start(tok_sb[:], d_tok[:])
            wq_sb = cst.tile([128, 4, DK], f32)
            nc.sync.dma_start(wq_sb[:], d_wq.rearrange("(c p) m -> p c m", p=128))
            wk_sb = cst.tile([128, 4, DK], f32)
            nc.sync.dma_start(wk_sb[:], d_wk.rearrange("(c p) m -> p c m", p=128))
            k0T_sb = cst.tile([128, 2, N], f32)
            nc.sync.dma_start(k0T_sb[:], d_k0T.rearrange("(c p) m -> p c m", p=128))
            wh_sb = cst.tile([128, 8, H], f32r)
            nc.sync.dma_start(wh_sb[:], d_wh.rearrange("(c p) m -> p c m", p=128))
            wvT_sb = cst.tile([128, 4, H], f32r)
            nc.sync.dma_start(wvT_sb[:], d_wvT.rearrange("(c p) m -> p c m", p=128))
            v0_sb = cst.tile([128, 4, DV], f32r)
            nc.sync.dma_start(v0_sb[:], d_v0.rearrange("(c p) m -> p c m", p=128))
            v0hT_sb = cst.tile([128, 4, T], f32r)
            nc.sync.dma_start(v0hT_sb[:], d_v0hT.rearrange("(c p) m -> p c m", p=128))
            bhb_sb = cst.tile([128, H], f32)
            nc.sync.dma_start(bhb_sb[:], d_bhb[:])
            maskRM_sb = cst.tile([128, RG, T], f32)
            nc.sync.dma_start(maskRM_sb[:], d_maskRM[:])
            negI_sb = cst.tile([T, T], f32r)
            nc.sync.dma_start(negI_sb[:], d_negI[:])
            eyer_sb = cst.tile([1, T, T], f32r)
            nc.sync.dma_start(eyer_sb[:], d_eyer.rearrange("p (t j) -> p t j", t=T))
            boutT_sb = cst.tile([128, VCH], f32)
            nc.sync.dma_start(boutT_sb[:], d_boutT[:])

            # persistent tensors for the scan
            big = stack.enter_context(tc.tile_pool(name="big", bufs=1))
            AT_sb = big.tile([T, TB], f32r)       # masked A^T  [slot, row]
            c_sb = big.tile([128, 4, H], f32r)    # C' = Wv @ Wh_r / B
            d0_sb = big.tile([T, H], f32r)
            gd_sb = big.tile([T, H], f32r)
            # no init needed: first read is at t=1, after step 0's mirror copy

            # =================== PHASE 0 ===================
            with contextlib.ExitStack() as ph0:
                xt_p = ph0.enter_context(tc.tile_pool(name="xt", bufs=1))
                p0 = ph0.enter_context(tc.tile_pool(name="p0", bufs=2))
                ps_mm = ph0.enter_context(
                    tc.tile_pool(name="ps_mm", bufs=4, space="PSUM"))
                ps_tr = ph0.enter_context(
                    tc.tile_pool(name="ps_tr", bufs=4, space="PSUM"))

                xT_sb = xt_p.tile([128, 4, TB], f32)      # fp32 (score path)
                xTr_sb = xt_p.tile([128, 4, TB], f32r)    # f32r (U path)
                xbT_sb = xt_p.tile([128, 4, T], f32)
                knT_sb = xt_p.tile([128, 2, T], f32)

                # --- pass A: gather X = emb[tok], transpose into xT ---
                for g in range(RG):
                    xg = p0.tile([128, E], f32, tag="xg")
                    nc.gpsimd.indirect_dma_start(
                        out=xg[:], out_offset=None, in_=d_emb[:],
                        in_offset=bass.IndirectOffsetOnAxis(
                            ap=tok_sb[:, g:g + 1], axis=0),
                    )
                    for e in range(4):
                        ptr = ps_tr.tile([128, 128], f32, tag="ptr")
                        nc.tensor.transpose(
                            out=ptr[:], in_=xg[:, e * 128:(e + 1) * 128],
                            identity=ident[:])
                        nc.scalar.copy(
                            out=xT_sb[:, e, g * 128:(g + 1) * 128], in_=ptr[:])
                        nc.vector.tensor_copy(
                            out=xTr_sb[:, e, g * 128:(g + 1) * 128], in_=ptr[:])

                # --- Xbar^T (batch sums; 1/B folded into Knew eviction) ---
                for e in range(4):
                    nc.vector.reduce_sum(
                        out=xbT_sb[:, e, :],
                        in_=xT_sb[:, e, :].rearrange("p (t b) -> p t b", b=B),
                        axis=mybir.AxisListType.X)

                # --- Knew^T = Wk^T Xbar^T / B  (fp32: key path) ---
                for m2 in range(2):
                    pk = ps_mm.tile([128, 512], f32, tag="pmm")
                    for e in range(4):
                        nc.tensor.matmul(
                            out=pk[:, 0:T],
                            lhsT=wk_sb[:, e, m2 * 128:(m2 + 1) * 128],
                            rhs=xbT_sb[:, e, :],
                            start=(e == 0), stop=(e == 3))
                    nc.scalar.activation(
                        out=knT_sb[:, m2, :], in_=pk[:, 0:T],
                        func=ACT.Copy, scale=float(1.0 / B))

                # --- pass B: per row-group full chain ---
                for g in range(RG):
                    gsl = slice(g * 128, (g + 1) * 128)

                    # Q^T columns for this group (fp32, scaled by 1/sqrt(DK))
                    qT = p0.tile([128, 2, 128], f32, tag="qT")
                    for m2 in range(2):
                        pq = ps_mm.tile([128, 512], f32, tag="pmm")
                        for e in range(4):
                            nc.tensor.matmul(
                                out=pq[:, 0:128],
                                lhsT=wq_sb[:, e, m2 * 128:(m2 + 1) * 128],
                                rhs=xT_sb[:, e, gsl],
                                start=(e == 0), stop=(e == 3))
                        nc.scalar.activation(
                            out=qT[:, m2, :], in_=pq[:, 0:128],
                            func=ACT.Copy, scale=float(1.0 / np.sqrt(DK)))

                    # scores (static + merged new)
                    s_g = p0.tile([128, N], f32, tag="sg")
                    ps_s = ps_mm.tile([128, N], f32, tag="pmm")
                    for k2 in range(2):
                        nc.tensor.matmul(
                            out=ps_s[:], lhsT=qT[:, k2, :], rhs=k0T_sb[:, k2, :],
                            start=(k2 == 0), stop=(k2 == 1))
                    nc.scalar.copy(out=s_g[:], in_=ps_s[:])
                    ps_n = ps_mm.tile([128, N], f32, tag="pmm")
                    for k2 in range(2):
                        nc.tensor.matmul(
                            out=ps_n[:, 0:T], lhsT=qT[:, k2, :],
                            rhs=knT_sb[:, k2, :],
                            start=(k2 == 0), stop=(k2 == 1))
                    nc.vector.copy_predicated(
                        out=s_g[:, 0:T],
                        mask=maskRM_sb[:, g, :].bitcast(mybir.dt.uint32),
                        data=ps_n[:, 0:T])

                    # top-8 threshold softmax -> scattered weights w_g
                    mx = p0.tile([128, 8], f32, tag="mx")
                    nc.vector.max(out=mx[:], in_=s_g[:])
                    negm1 = p0.tile([128, 1], f32, tag="negm1")
                    nc.vector.tensor_scalar_mul(negm1[:], mx[:, 0:1], -1.0)
                    emx = p0.tile([128, 8], f32, tag="emx")
                    nc.scalar.activation(out=emx[:], in_=mx[:], func=ACT.Exp,
                                         bias=negm1[:, 0:1])
                    zrow = p0.tile([128, 1], f32, tag="zrow")
                    nc.vector.reduce_sum(out=zrow[:], in_=emx[:],
                                         axis=mybir.AxisListType.X)
                    winv = p0.tile([128, 1], f32, tag="winv")
                    nc.vector.reciprocal(out=winv[:], in_=zrow[:])
                    eb = p0.tile([128, N], f32, tag="eb")
                    nc.scalar.activation(out=eb[:], in_=s_g[:], func=ACT.Exp,
                                         bias=negm1[:, 0:1])
                    w_g = p0.tile([128, N], f32, tag="wg")
                    nc.vector.scalar_tensor_tensor(
                        out=w_g[:], in0=s_g[:], scalar=mx[:, 7:8], in1=eb[:],
                        op0=mybir.AluOpType.is_ge, op1=mybir.AluOpType.mult)
                    nc.vector.tensor_scalar_mul(w_g[:], w_g[:], winv[:, 0:1])

                    if _DEBUG:
                        nc.sync.dma_start(d_dbg_s[:, g, :], s_g[:])
                        nc.sync.dma_start(d_dbg_w[:, g, :], w_g[:])

                    # A^T columns: mask (slot<t) then transpose [128,64]->[64,128]
                    am = p0.tile([128, T], f32, tag="am")
                    nc.vector.tensor_mul(am[:], w_g[:, 0:T], maskRM_sb[:, g, :])
                    pat = ps_tr.tile([128, 128], f32, tag="ptr")
                    nc.tensor.transpose(out=pat[0:T, :], in_=am[:],
                                        identity=ident[:])
                    nc.scalar.copy(out=AT_sb[:, gsl], in_=pat[0:T, :])

                    # Wfull^T columns
                    wfT = p0.tile([128, 4, 128], f32r, tag="wfT")
                    for s4 in range(4):
                        ptr = ps_tr.tile([128, 128], f32, tag="ptr")
                        nc.tensor.transpose(
                            out=ptr[:], in_=w_g[:, s4 * 128:(s4 + 1) * 128],
                            identity=ident[:])
                        nc.scalar.copy(out=wfT[:, s4, :], in_=ptr[:])

                    # R^T columns = values0^T @ Wfull^T   (f32r)
                    rT = p0.tile([128, 4, 128], f32r, tag="rT")
                    for m4 in range(4):
                        pr = ps_mm.tile([128, 512], f32, tag="pmm")
                        for s4 in range(4):
                            nc.tensor.matmul(
                                out=pr[:, 0:128],
                                lhsT=v0_sb[:, s4, m4 * 128:(m4 + 1) * 128],
                                rhs=wfT[:, s4, :],
                                start=(s4 == 0), stop=(s4 == 3))
                        nc.scalar.copy(out=rT[:, m4, :], in_=pr[:, 0:128])

                    # U rows for this group = [X|R] @ Wh + bh   (f32r)
                    pu = ps_mm.tile([128, H], f32, tag="pmm")
                    for e in range(4):
                        nc.tensor.matmul(
                            out=pu[:], lhsT=xTr_sb[:, e, gsl],
                            rhs=wh_sb[:, e, :], start=(e == 0), stop=False)
                    for d4 in range(4):
                        nc.tensor.matmul(
                            out=pu[:], lhsT=rT[:, d4, :],
                            rhs=wh_sb[:, 4 + d4, :], start=False,
                            stop=(d4 == 3))
                    ub = p0.tile([128, H], f32r, tag="ub")
                    nc.vector.tensor_add(out=ub[:], in0=pu[:], in1=bhb_sb[:])
                    nc.sync.dma_start(
                        d_ub.rearrange("t b h -> (t b) h")[gsl], ub[:])

                # --- C' = Wv @ Wh_r / B ;  D0 = values0[:64] @ Wh_r ---
                for m4 in range(4):
                    pc = ps_mm.tile([128, H], f32, tag="pmm")
                    for d4 in range(4):
                        nc.tensor.matmul(
                            out=pc[:],
                            lhsT=wvT_sb[:, d4, m4 * 128:(m4 + 1) * 128],
                            rhs=wh_sb[:, 4 + d4, :], start=(d4 == 0),
                            stop=(d4 == 3))
                    nc.scalar.activation(out=c_sb[:, m4, :], in_=pc[:],
                                         func=ACT.Copy, scale=float(1.0 / B))
                pd = ps_mm.tile([128, H], f32, tag="pmm")
                for d4 in range(4):
                    nc.tensor.matmul(
                        out=pd[0:T, :], lhsT=v0hT_sb[:, d4, :],
                        rhs=wh_sb[:, 4 + d4, :], start=(d4 == 0),
                        stop=(d4 == 3))
                nc.scalar.copy(out=d0_sb[:], in_=pd[0:T, :])

            # hT allocated after phase-0 transients are freed
            hTp = stack.enter_context(tc.tile_pool(name="hTp", bufs=1))
            hT_sb = hTp.tile([128, 4, TB], f32r)

            # =================== PHASE 1: the scan ===================
            with contextlib.ExitStack() as ph1:
                ps_z = ph1.enter_context(
                    tc.tile_pool(name="ps_z", bufs=2, space="PSUM"))
                ps_t = ph1.enter_context(
                    tc.tile_pool(name="ps_t", bufs=3, space="PSUM"))
                ps_g = ph1.enter_context(
                    tc.tile_pool(name="ps_g", bufs=1, space="PSUM"))
                ps_gd = ph1.enter_context(
                    tc.tile_pool(name="ps_gd", bufs=1, space="PSUM"))
                sc = ph1.enter_context(tc.tile_pool(name="sc", bufs=2))
                up = ph1.enter_context(tc.tile_pool(name="up", bufs=8))

                # Gd accumulates in PSUM via rank-1 row placements; mirrored
                # to SBUF (gd_sb) with an aligned full-tile copy each step.
                psum_gd = ps_gd.tile([T, H], f32, tag="gdm")

                for t in range(T):
                    u_t = up.tile([B, H], f32r, tag="ut")
                    nc.sync.dma_start(u_t[:], d_ub[t])
                    pz = ps_z.tile([B, H], f32, tag="pz")
                    nc.tensor.matmul(out=pz[:], lhsT=identr[:], rhs=u_t[:],
                                     start=True, stop=(t == 0))
                    if t > 0:       # A_0 == 0, and gd_sb is first written at t=0
                        nc.tensor.matmul(
                            out=pz[:], lhsT=AT_sb[:, t * B:(t + 1) * B],
                            rhs=gd_sb[:], start=False, stop=True)
                    h_t = sc.tile([B, H], f32, tag="ht")
                    nc.scalar.activation(out=h_t[:], in_=pz[:], func=ACT.Tanh)
                    hbT = sc.tile([128, 4], f32r, tag="hbT")
                    for c4 in range(4):
                        pt = ps_t.tile([128, B], f32, tag="pt")
                        nc.tensor.transpose(
                            out=pt[:], in_=h_t[:, c4 * 128:(c4 + 1) * 128],
                            identity=ident[0:B, 0:B])
                        nc.scalar.copy(
                            out=hT_sb[:, c4, t * B:(t + 1) * B], in_=pt[:])
                        with nc.allow_low_precision(
                                reason="batch-sum output rounded to f32r "
                                       "for the PE; accumulator is fp32"):
                            nc.vector.reduce_sum(
                                out=hbT[:, c4:c4 + 1], in_=pt[:],
                                axis=mybir.AxisListType.X)
                    pg = ps_g.tile([1, H], f32, tag="pg")
                    for c4 in range(4):
                        nc.tensor.matmul(
                            out=pg[:], lhsT=hbT[:, c4:c4 + 1],
                            rhs=c_sb[:, c4, :], start=(c4 == 0), stop=False)
                    nc.tensor.matmul(
                        out=pg[:], lhsT=negI_sb[:, t:t + 1], rhs=d0_sb[:],
                        start=False, stop=True)
                    stage = sc.tile([1, H], f32r, tag="stg")
                    nc.scalar.copy(out=stage[:], in_=pg[:])
                    nc.tensor.matmul(
                        out=psum_gd[:], lhsT=eyer_sb[:, t, :], rhs=stage[:],
                        start=(t == 0), stop=(t == T - 1),
                        skip_group_check=True)
                    nc.scalar.copy(out=gd_sb[:], in_=psum_gd[:])

            if _DEBUG:
                nc.sync.dma_start(d_dbg_hT[:], hT_sb[:].bitcast(f32))
                nc.sync.dma_start(d_dbg_at[:], AT_sb[:].bitcast(f32))
                nc.sync.dma_start(d_dbg_gd[:], gd_sb[:].bitcast(f32))

            # =============== PHASE 2: logits^T = Wout^T h^T ===============
            with contextlib.ExitStack() as ph2:
                wo_p = ph2.enter_context(tc.tile_pool(name="wo", bufs=3))
                ps_o = ph2.enter_context(
                    tc.tile_pool(name="ps_o", bufs=4, space="PSUM"))
                ob_p = ph2.enter_context(tc.tile_pool(name="ob", bufs=4))

                for vc in range(VCH):
                    vsz = 128 if vc < VCH - 1 else VLAST
                    wo = wo_p.tile([128, 4, 128], f32r, tag="wo")
                    nc.sync.dma_start(wo[:], d_wout[vc].rearrange("c k m -> k c m"))
                    for n4 in range(4):
                        po = ps_o.tile([128, 512], f32, tag="po")
                        for hc in range(4):
                            nc.tensor.matmul(
                                out=po[:],
                                lhsT=wo[:, hc, :],
                                rhs=hT_sb[:, hc, n4 * 512:(n4 + 1) * 512],
                                start=(hc == 0), stop=(hc == 3))
                        ob = ob_p.tile([128, 512], f32, tag="ob")
                        nc.vector.tensor_scalar_add(
                            ob[0:vsz, :], po[0:vsz, :],
                            boutT_sb[0:vsz, vc:vc + 1])
                        nc.sync.dma_start(
                            d_out[vc * 128:vc * 128 + vsz,
                                  n4 * 512:(n4 + 1) * 512],
                            ob[0:vsz, :])

    nc.compile()
    return nc


_CACHE = {}


def _get_program():
    if "nc" not in _CACHE:
        _CACHE["nc"] = _build_program()
    return _CACHE["nc"]


def _host_prep(tokens, emb, Wq, Wk, Wv, Wh, bh, Wout, bout, keys0, values0):
    tok = np.ascontiguousarray(
        np.asarray(tokens, np.int64).reshape(TB).astype(np.int32))
    tok_cm = np.zeros((128, RG), np.int32)
    for g in range(RG):
        tok_cm[:, g] = tok[g * 128:(g + 1) * 128]

    t_of_row = np.repeat(np.arange(T), B)                      # [TB]
    maskRM = (np.arange(T)[None, :] < t_of_row[:, None]).astype(np.float32)
    maskRM_cm = np.zeros((128, RG, T), np.float32)
    for g in range(RG):
        maskRM_cm[:, g, :] = maskRM[g * 128:(g + 1) * 128]

    base = {
        "tok": tok_cm,
        "emb": np.ascontiguousarray(np.asarray(emb, np.float32)),
        "wq": np.ascontiguousarray(np.asarray(Wq, np.float32)),
        "wk": np.ascontiguousarray(np.asarray(Wk, np.float32)),
        "k0T": np.ascontiguousarray(np.asarray(keys0, np.float32).T),
        "wh": _round_f32r(np.asarray(Wh, np.float32)),
        "wvT": _round_f32r(np.asarray(Wv, np.float32).T),
        "v0": _round_f32r(np.asarray(values0, np.float32)),
        "v0hT": _round_f32r(np.asarray(values0, np.float32)[:T].T),
        "bhb": np.ascontiguousarray(
            np.broadcast_to(np.asarray(bh, np.float32), (128, H))),
        "maskRM": maskRM_cm,
        "negI": np.ascontiguousarray(-np.eye(T, dtype=np.float32)),
        "eyer": np.ascontiguousarray(
            np.eye(T, dtype=np.float32).reshape(1, T * T)),
    }

    Wout = np.asarray(Wout, np.float32)
    bout = np.asarray(bout, np.float32)
    in_maps = []
    for c in range(NCORES):
        wsh = Wout[:, c * VSH:(c + 1) * VSH]
        wt = np.zeros((VCH, 4, 128, 128), np.float32)
        for vc in range(VCH):
            vsz = 128 if vc < VCH - 1 else VLAST
            for hc in range(4):
                wt[vc, hc, :, :vsz] = \
                    wsh[hc * 128:(hc + 1) * 128, vc * 128:vc * 128 + vsz]
        bt = np.zeros((128, VCH), np.float32)
        bsh = bout[c * VSH:(c + 1) * VSH]
        for vc in range(VCH):
            vsz = 128 if vc < VCH - 1 else VLAST
            bt[:vsz, vc] = bsh[vc * 128:vc * 128 + vsz]
        in_maps.append({**base, "woutc": _round_f32r(wt), "boutc": bt})
    return in_maps


def run_on_device(in_maps, trace=False):
    from concourse import bass_utils
    nc = _get_program()
    return bass_utils.run_bass_kernel_spmd(
        nc, in_maps, core_ids=list(range(NCORES)), trace=trace)


def kernel(tokens, emb, Wq, Wk, Wv, Wh, bh, Wout, bout, keys0, values0, k):
    assert int(k) == K
    in_maps = _host_prep(tokens, emb, Wq, Wk, Wv, Wh, bh, Wout, bout,
                         keys0, values0)
    res = run_on_device(in_maps)
    parts = [res.results[c]["out"] for c in range(NCORES)]       # each [VSH, TB]
    logitsT = np.concatenate(parts, axis=0)                      # [V, TB]
    return np.ascontiguousarray(logitsT.T).reshape(T, B, V)
